# revision 34
# baseline (speedup 1.0000x reference)
"""Trainium2 Bass kernel for single-"head" LlamaAttention.

Reference computation (per batch b):
    q = hs @ Wq.T ; k = hs @ Wk.T ; v = hs @ Wv.T          # [S, H]
    scores = (q @ k.T) / sqrt(128) + mask                  # [S, S]
    probs  = softmax(scores, axis=-1)
    out    = (probs @ v) @ Wo.T                            # [S, H]

Shapes: B=2, S=4096, H=2048, fp32 I/O.

Sharding: 8 cores, 4 per batch element; each core owns 1024 query rows,
which are also its shard of 1024 keys.  Each core projects K/V only for its
own shard, then an AllGather within each 4-core group assembles the full
per-batch K^T and V.

Schedule (all matmuls bf16 with fp32 PSUM accumulation):
  B:   KT shard  = Wk proj of own key columns       -> kt_p (DRAM)
  CC1: AllGather kt_p -> kt_g   (issued right after B; overlaps C+A+D1-own)
  C:   V shard   = Wv proj                          -> v_p (DRAM)
  CC2: AllGather v_p -> v_g     (queues behind CC1; overlaps A+D1)
  A:   QT        = Wq proj, written directly to SBUF qt (no DRAM roundtrip)
  D1 (for BOTH query halves, before any D3): S^T chunks = KT.T @ QT,
      P^T = exp(scale*S^T + mask^T) -> spilled to pt_d (DRAM),
      l[q] = ones.T @ P^T accumulated on PE (2 blocks behind exp)
  D2:  rl = 1/l, broadcast to 128 partitions via K=1 matmul
  D3 (per half): ctx^T = sum_k V-tiles.T @ P^T-tiles (streamed back),
      multiplied by rl on PSUM->SBUF
  D4 (per half): O = ctx^T-chunks.T @ WoT -> output

Hoisting both D1 rounds ahead of D3 keeps PE busy while the V AllGather
(which serializes behind the K AllGather on the collective cores) is in
flight; P^T spills to DRAM because SBUF cannot hold both halves' P^T
alongside the weights.
"""

import math

import numpy as np

import sys

sys.path.insert(0, "/opt/trn_rl_repo")

import ml_dtypes  # noqa: E402

import concourse.bass as bass  # noqa: E402
import concourse.tile as tile  # noqa: E402
from concourse import bacc, mybir  # noqa: E402
from concourse.ap import AP as AAP  # noqa: E402
from concourse.bass_utils import run_bass_kernel_spmd  # noqa: E402

BF16 = mybir.dt.bfloat16
F32 = mybir.dt.float32
NP_BF16 = ml_dtypes.bfloat16


class Cfg:
    def __init__(self, S=4096, H=2048, QPC=1024, head_dim=128):
        self.S = S          # keys per batch
        self.H = H          # hidden
        self.QPC = QPC      # queries per core (also keys per core shard)
        self.GPC = 4        # cores per batch group
        self.HC = H // 128  # contraction chunks
        self.KB = S // 128  # key blocks
        self.NH = 512       # free-dim tile
        self.QR = min(QPC, 512)  # queries per round
        self.NQG = QPC // self.QR
        self.scale = 1.0 / math.sqrt(head_dim)
        assert QPC % self.NH == 0 and S == self.GPC * QPC


def build_nc(cfg: Cfg) -> bass.Bass:
    S, H, QPC = cfg.S, cfg.H, cfg.QPC
    HC, KB, NH, QR, NQG, GPC = cfg.HC, cfg.KB, cfg.NH, cfg.QR, cfg.NQG, cfg.GPC

    nc = bacc.Bacc(None, target_bir_lowering=False, num_devices=2 * GPC)

    hsq = nc.dram_tensor("hsq", [H, QPC], BF16, kind="ExternalInput")
    wqT = nc.dram_tensor("wqt", [H, H], BF16, kind="ExternalInput")
    wkT = nc.dram_tensor("wkt", [H, H], BF16, kind="ExternalInput")
    wvT = nc.dram_tensor("wvt", [H, H], BF16, kind="ExternalInput")
    woT = nc.dram_tensor("wot", [H, H], BF16, kind="ExternalInput")
    maskT = nc.dram_tensor("maskt", [S, QPC], BF16, kind="ExternalInput")
    o_out = nc.dram_tensor("o", [QPC, H], F32, kind="ExternalOutput")

    kt_p = nc.dram_tensor("kt_p", [HC, 128, QPC], BF16)
    v_p = nc.dram_tensor("v_p", [QPC, H], BF16)
    kt_g = nc.dram_tensor("kt_g", [GPC, HC, 128, QPC], BF16)
    v_g = nc.dram_tensor("v_g", [GPC, QPC, H], BF16)
    pt_d = nc.dram_tensor("pt_d", [NQG, KB, 128, QR], BF16)
    cto_d = nc.dram_tensor("cto_d", [NQG, HC, 128, QR], BF16)

    groups = [list(range(g * GPC, (g + 1) * GPC)) for g in range(2)]
    mm = mybir.AluOpType.mult

    with tile.TileContext(nc) as tc:
        with (
            tc.tile_pool(name="hs", bufs=1) as hpool,
            tc.tile_pool(name="w", bufs=1) as wpool,
            tc.tile_pool(name="qt", bufs=1) as qtpool,
            tc.tile_pool(name="stream", bufs=3) as spool,
            tc.tile_pool(name="ptw", bufs=4) as ptwpool,
            tc.tile_pool(name="ptown", bufs=8) as ptopool,
            tc.tile_pool(name="ptr", bufs=3) as ptrpool,
            tc.tile_pool(name="ct", bufs=1) as ctpool,
            tc.tile_pool(name="mk", bufs=2) as mkpool,
            tc.tile_pool(name="v", bufs=2) as vpool,
            tc.tile_pool(name="stg", bufs=2) as stpool,
            tc.tile_pool(name="stgf", bufs=2) as stfpool,
            tc.tile_pool(name="misc", bufs=1) as mpool,
            tc.tile_pool(name="ps", bufs=8, space="PSUM") as pspool,
        ):
            # Weights live in two half-column slots (w0/w1, 32KB/part each)
            # so the NEXT phase's weight half can stream in while the current
            # phase is still reading the other half — with a single 64KB slot
            # every weight load serializes behind the previous phase's last
            # read, costing ~25us per phase boundary.
            HH = H // 2

            def load_w(dram, half, eng=None):
                # Mid-phase weight refills dispatch on the Activation HWDGE
                # queue so they don't delay the SP queue's store traffic
                # (which gates the K AllGather).
                t = wpool.tile([128, HC, HH], BF16, tag=f"w{half}",
                               name=f"w{half}")
                (eng or nc.sync).dma_start(
                    out=t[:],
                    in_=dram[:, half * HH:(half + 1) * HH].rearrange(
                        "(c p) o -> p c o", p=128
                    ),
                )
                return t

            # hsq resident: serves K/V shard projections and Q projection.
            # Loaded in halves, interleaved with the wk halves, so phase B's
            # first matmul fires ~18us in instead of ~38us.
            hq = hpool.tile([128, HC, QPC], BF16, tag="hs")
            nc.sync.dma_start(
                out=hq[:, :, 0:NH],
                in_=hsq[:, 0:NH].rearrange("(c p) q -> p c q", p=128),
            )
            # First two wk column blocks ASAP so matmuls start ~10us in; the
            # rest of wk and hq follow while phase B runs.
            wk0 = wpool.tile([128, HC, HH], BF16, tag="w0", name="w0")
            nc.sync.dma_start(
                out=wk0[:, :, 0:256],
                in_=wkT[:, 0:256].rearrange("(c p) o -> p c o", p=128),
            )
            nc.sync.dma_start(
                out=hq[:, :, NH:QPC],
                in_=hsq[:, NH:QPC].rearrange("(c p) q -> p c q", p=128),
            )
            nc.sync.dma_start(
                out=wk0[:, :, 256:HH],
                in_=wkT[:, 256:HH].rearrange("(c p) o -> p c o", p=128),
            )
            wk = [wk0, load_w(wkT, 1)]

            # ---------- Phase B: KT shard (this core's QPC keys) ----------
            # hb outer so the first half of wk suffices to start.
            for hb in range(HC):
                for kcb in range(QPC // NH):
                    ps = pspool.tile([128, NH], F32, tag="ps")
                    for hc in range(HC):
                        nc.tensor.matmul(
                            ps[:],
                            wk[hb // 8][:, hc, (hb % 8) * 128:(hb % 8 + 1) * 128],
                            hq[:, hc, kcb * NH:(kcb + 1) * NH],
                            start=(hc == 0),
                            stop=(hc == HC - 1),
                        )
                    st = stpool.tile([128, NH], BF16, tag="stg")
                    nc.scalar.copy(st[:], ps[:])
                    nc.scalar.dma_start(
                        out=kt_p[hb, :, kcb * NH:(kcb + 1) * NH], in_=st[:]
                    )
                if hb == 7:
                    # wk half 0 is dead; its slot can take wv half 0 now.
                    wv = [load_w(wvT, 0, nc.scalar)]

            # KT gather issued immediately: D1 consumes kt_g first, and this
            # gives the collective phases C+A of PE work to hide under.
            nc.gpsimd.collective_compute(
                "AllGather",
                mybir.AluOpType.bypass,
                replica_groups=groups,
                ins=[kt_p[:]],
                outs=[kt_g[:]],
            )
            wv.append(load_w(wvT, 1, nc.scalar))

            # ---------- Phase C: V shard (hh outer: half 0 of wv suffices
            # to start; wq halves refill the slots as wv halves go dead) ----
            for hh in range(H // NH):
                for kcb in range(QPC // NH):
                    for kb4 in range(NH // 128):
                        ps = pspool.tile([128, NH], F32, tag="ps")
                        for hc in range(HC):
                            nc.tensor.matmul(
                                ps[:],
                                hq[:, hc, kcb * NH + kb4 * 128: kcb * NH + (kb4 + 1) * 128],
                                wv[hh // 2][:, hc, (hh % 2) * NH:(hh % 2 + 1) * NH],
                                start=(hc == 0),
                                stop=(hc == HC - 1),
                            )
                        st = stpool.tile([128, NH], BF16, tag="stg")
                        nc.scalar.copy(st[:], ps[:])
                        nc.scalar.dma_start(
                            out=v_p[
                                kcb * NH + kb4 * 128: kcb * NH + (kb4 + 1) * 128,
                                hh * NH:(hh + 1) * NH,
                            ],
                            in_=st[:],
                        )
                if hh == 1:
                    wq = [load_w(wqT, 0, nc.scalar)]
            wq.append(load_w(wqT, 1, nc.scalar))

            # ---------- Phase A: QT projection, directly into SBUF --------
            qt = qtpool.tile([128, HC, QPC], BF16, tag="qt")
            for hb in range(HC):
                for qg in range(NQG):
                    ps = pspool.tile([128, QR], F32, tag="ps")
                    for hc in range(HC):
                        nc.tensor.matmul(
                            ps[:],
                            wq[hb // 8][:, hc, (hb % 8) * 128:(hb % 8 + 1) * 128],
                            hq[:, hc, qg * QR:(qg + 1) * QR],
                            start=(hc == 0),
                            stop=(hc == HC - 1),
                        )
                    nc.scalar.copy(qt[:, hb, qg * QR:(qg + 1) * QR], ps[:])
                if hb == 7:
                    wo = [load_w(woT, 0, nc.scalar)]
            wo.append(load_w(woT, 1, nc.scalar))

            # ---------- D1 for BOTH query halves (overlaps V gather) ------
            # Key-processing order per core: [own shard, peer pid^1, pid^2,
            # pid^3].  The own shard reads kt_p / v_p (local, no collective
            # dependency), so D1 starts the moment phase A ends instead of
            # stalling on the K AllGather.  Peer slots of kt_g / v_g are
            # addressed with runtime (partition_id ^ j) offsets so the same
            # SPMD program skips its own slot uniformly.  The host permutes
            # each core's mask columns to match this key order.
            ones_col = mpool.tile([128, 1], BF16, tag="m_ones")
            nc.vector.memset(ones_col[:], 1.0)
            ones_row = mpool.tile([1, 128], F32, tag="m_onesr")
            nc.vector.memset(ones_row[:], 1.0)

            XOR = mybir.AluOpType.bitwise_xor
            ADD = mybir.AluOpType.add
            MUL = mybir.AluOpType.mult
            # kt_g / v_g slots are indexed by within-group rank, so mask the
            # global partition id down to rank = pid & (GPC-1) before XOR.
            pid = nc.sync.partition_id()
            AND = mybir.AluOpType.bitwise_and
            rot_k, rot_v = [None], [None]
            for j in range(1, GPC):
                rk = nc.sync.alloc_register(f"rot_k{j}")
                nc.sync.reg_alu(rk, pid, j, XOR)
                nc.sync.reg_alu(rk, rk, GPC - 1, AND)
                nc.sync.reg_alu(rk, rk, HC * 128 * QPC, MUL)
                rot_k.append(rk)
                rv = nc.sync.alloc_register(f"rot_v{j}")
                nc.sync.reg_alu(rv, pid, j, XOR)
                nc.sync.reg_alu(rv, rv, GPC - 1, AND)
                nc.sync.reg_alu(rv, rv, QPC * H, MUL)
                rot_v.append(rv)

            KL = 256  # keys per kt load
            CPS = KL // 128  # kb blocks per chunk
            KPS = QPC // KL  # chunks per shard
            rbs = []
            l_pss = [pspool.tile([1, QR], F32, tag="ps", name=f"l_ps{qg}")
                     for qg in range(NQG)]
            pt_wins = [[None] * KB for _ in range(NQG)]
            l_next = [0] * NQG  # next key block to fold into l, per half

            def flush_l(qg, upto):
                while l_next[qg] < upto:
                    kb = l_next[qg]
                    nc.tensor.matmul(
                        l_pss[qg][:],
                        ones_col[:, 0:1],
                        pt_wins[qg][kb][:],
                        start=(kb == 0),
                        stop=(kb == KB - 1),
                    )
                    if kb >= QPC // 128:
                        pt_wins[qg][kb] = None
                    l_next[qg] += 1

            def emit_d1_chunk(qg, j, ch):
                qsl = slice(qg * QR, (qg + 1) * QR)
                lo = ch * KL
                ktq = spool.tile([128, HC, KL], BF16, tag="stream",
                                 name="ktq")
                if j == 0:
                    nc.sync.dma_start(
                        out=ktq[:],
                        in_=kt_p[:, :, lo:lo + KL].rearrange("c p k -> p c k"),
                    )
                else:
                    base = kt_g[0, :, :, lo:lo + KL].rearrange("c p k -> p c k")
                    ro = nc.sync.alloc_register(f"ko{qg}_{j}_{ch}")
                    nc.sync.reg_alu(ro, rot_k[j], lo, ADD)
                    nc.sync.dma_start(
                        out=ktq[:], in_=AAP(base.tensor, ro, base.ap)
                    )
                kbq = j * KPS + ch  # chunk index in processing order
                mk = mkpool.tile([128, CPS, QR], BF16, tag="mk", name="mk")
                nc.scalar.dma_start(
                    out=mk[:],
                    in_=maskT[kbq * KL:(kbq + 1) * KL, qsl].rearrange(
                        "(b p) q -> p b q", p=128
                    ),
                )
                for kb4 in range(CPS):
                    kb = kbq * CPS + kb4
                    ps = pspool.tile([128, QR], F32, tag="ps", name="ps")
                    for hc in range(HC):
                        nc.tensor.matmul(
                            ps[:],
                            ktq[:, hc, kb4 * 128:(kb4 + 1) * 128],
                            qt[:, hc, qsl],
                            start=(hc == 0),
                            stop=(hc == HC - 1),
                        )
                    tmp = stfpool.tile([128, QR], F32, tag="stgf", name="tmp")
                    nc.vector.scalar_tensor_tensor(
                        out=tmp[:],
                        in0=ps[:],
                        scalar=cfg.scale,
                        in1=mk[:, kb4, :],
                        op0=mm,
                        op1=mybir.AluOpType.add,
                    )
                    if j == 0:
                        # Own-shard P^T stays in SBUF: D3-own consumes it
                        # directly (no DRAM round-trip), and the peer pass
                        # never needs it again.
                        ptw = ptopool.tile([128, QR], BF16, tag="ptown",
                                           name="ptw_o")
                    else:
                        ptw = ptwpool.tile([128, QR], BF16, tag="ptw",
                                           name="ptw")
                    nc.scalar.activation(
                        out=ptw[:], in_=tmp[:],
                        func=mybir.ActivationFunctionType.Exp,
                    )
                    pt_wins[qg][kb] = ptw
                    if j > 0:
                        nc.scalar.dma_start(out=pt_d[qg, kb], in_=ptw[:])
                    flush_l(qg, kb - 1)  # stay 2 behind the exp writes

            def emit_d3_own(qg):
                # ctx^T partial over the LOCAL V shard — gather-free, consumes
                # the own-shard P^T tiles straight from SBUF.  The raw partial
                # spills to DRAM (PSUM can't stay resident through the gather
                # wait) and is folded back in during the peer pass.
                for ho4 in range(HC // 4):
                    cps = [
                        pspool.tile([128, QR], F32, tag="ps", name=f"ops{i}")
                        for i in range(4)
                    ]
                    for kb in range(QPC // 128):
                        lr = kb * 128
                        vt = vpool.tile([128, 4, 128], BF16, tag="v", name="vt")
                        nc.sync.dma_start(
                            out=vt[:],
                            in_=v_p[
                                lr:lr + 128, ho4 * 512:(ho4 + 1) * 512
                            ].rearrange("p (b h) -> p b h", b=4),
                        )
                        for i4 in range(4):
                            nc.tensor.matmul(
                                cps[i4][:],
                                vt[:, i4, :],
                                pt_wins[qg][kb][:],
                                start=(kb == 0),
                                stop=(kb == QPC // 128 - 1),
                            )
                    for i4 in range(4):
                        st = stpool.tile([128, QR], BF16, tag="stg", name="sto")
                        nc.scalar.copy(st[:], cps[i4][:])
                        nc.scalar.dma_start(
                            out=cto_d[qg, ho4 * 4 + i4], in_=st[:]
                        )

            # Pass 1: own-shard scores + own-V context partials for BOTH query
            # halves — all local, no gather dependency — emitted first so PSUM
            # slot chains don't tie any of it behind gather-blocked work.
            for qg in range(NQG):
                for ch in range(KPS):
                    emit_d1_chunk(qg, 0, ch)
                flush_l(qg, QPC // 128)  # finish own-shard l before pass 2
                emit_d3_own(qg)

            # Pass 2: peer-shard scores (needs the K gather), then D2.
            for qg in range(NQG):
                for j in range(1, GPC):
                    for ch in range(KPS):
                        emit_d1_chunk(qg, j, ch)
                flush_l(qg, KB)

                # --- D2: reciprocal, then transpose 1/l into per-partition
                # [128,1] scalars (one per query block) so D4 can fold the
                # divide into its PSUM drain with a single tensor_scalar_mul.
                l_sb = mpool.tile([1, QR], F32, tag="m_l")
                nc.scalar.copy(l_sb[:], l_pss[qg][:])
                rl = mpool.tile([1, QR], F32, tag="m_rl")
                nc.vector.reciprocal(rl[:], l_sb[:])
                rlts = []
                for qb in range(QR // 128):
                    rlt = mpool.tile([128, 1], F32, tag="m_rlt", bufs=8,
                                     name="rlt")
                    nc.sync.dma_start(
                        out=rlt[:],
                        in_=rl[0:1, qb * 128:(qb + 1) * 128].rearrange(
                            "a q -> q a"
                        ),
                    )
                    rlts.append(rlt)
                rbs.append(rlts)

            # ---------- V AllGather ----------
            # Emitted AFTER the D1 loops: DMA instructions conservatively
            # serialize behind collectives that precede them in program order,
            # so issuing this any earlier stalls D1's kt_g/mask streams until
            # the gather lands.  Its input dep (v_p stores) cleared long ago,
            # and on the collective queue it still starts right after the KT
            # gather finishes.
            nc.gpsimd.collective_compute(
                "AllGather",
                mybir.AluOpType.bypass,
                replica_groups=groups,
                ins=[v_p[:]],
                outs=[v_g[:]],
            )

            # ---------- D3 + D4 per query half ----------
            # tile_wait_until pushes everything below past the V AllGather in
            # the scheduler's virtual clock.  Without it the scheduler
            # interleaves D3's v_g loads ahead of D1's pt_d stores on the
            # strict-FIFO DMA queues, stalling D1 on the gather for ~200us.
            for qg in range(NQG):
                with tc.tile_wait_until(0.64 + 0.20 * qg):
                    rlts = rbs[qg]
                    # --- D3-peer: accumulate the 3 gathered V shards, then
                    # fold in the spilled own-shard partial and divide ---
                    ct = ctpool.tile([128, HC, QR], BF16, tag="ct")
                    for ho in range(HC // 8):
                        cps = [
                            pspool.tile([128, QR], F32, tag="ps", name=f"cps{i}")
                            for i in range(8)
                        ]
                        # Prefetch this group's own-context partials up front:
                        # the folds fire the moment each accumulator stops,
                        # instead of queueing readbacks behind the ptr stream.
                        ctos = []
                        for i8 in range(8):
                            cto = ptrpool.tile([128, QR], BF16, tag="cto",
                                               bufs=6, name="cto")
                            nc.scalar.dma_start(
                                out=cto[:], in_=cto_d[qg, ho * 8 + i8]
                            )
                            ctos.append(cto)
                        first = True
                        for j in range(1, GPC):
                            for kb8 in range(QPC // 128):
                                kb = j * (QPC // 128) + kb8
                                lr = kb8 * 128
                                vt = vpool.tile([128, 8, 128], BF16, tag="v")
                                base = v_g[
                                    0, lr:lr + 128, ho * 1024:(ho + 1) * 1024
                                ].rearrange("p (b h) -> p b h", b=8)
                                ro = nc.sync.alloc_register(
                                    f"vo{qg}_{ho}_{j}_{kb8}"
                                )
                                nc.sync.reg_alu(
                                    ro, rot_v[j], lr * H + ho * 1024, ADD
                                )
                                nc.sync.dma_start(
                                    out=vt[:], in_=AAP(base.tensor, ro, base.ap)
                                )
                                ptr = ptrpool.tile([128, QR], BF16, tag="ptr")
                                nc.scalar.dma_start(out=ptr[:], in_=pt_d[qg, kb])
                                for i8 in range(8):
                                    nc.tensor.matmul(
                                        cps[i8][:],
                                        vt[:, i8, :],
                                        ptr[:],
                                        start=first,
                                        stop=(j == GPC - 1)
                                        and (kb8 == QPC // 128 - 1),
                                    )
                                first = False
                        for i8 in range(8):
                            nc.vector.tensor_add(
                                out=ct[:, ho * 8 + i8, :], in0=cps[i8][:],
                                in1=ctos[i8][:],
                            )

                    # --- D4: output projection ---
                    for qb in range(QR // 128):
                        for hh in range(H // NH):
                            ps = pspool.tile([128, NH], F32, tag="ps")
                            for hc in range(HC):
                                nc.tensor.matmul(
                                    ps[:],
                                    ct[:, hc, qb * 128:(qb + 1) * 128],
                                    wo[hh // 2][:, hc, (hh % 2) * NH:(hh % 2 + 1) * NH],
                                    start=(hc == 0),
                                    stop=(hc == HC - 1),
                                )
                            ob = stfpool.tile([128, NH], F32, tag="stgf")
                            nc.scalar.activation(
                                out=ob[:], in_=ps[:],
                                func=mybir.ActivationFunctionType.Copy,
                                scale=rlts[qb],
                            )
                            nc.scalar.dma_start(
                                out=o_out[
                                    qg * QR + qb * 128: qg * QR + (qb + 1) * 128,
                                    hh * NH:(hh + 1) * NH,
                                ],
                                in_=ob[:],
                            )
    nc.finalize()
    return nc


def make_in_maps(cfg: Cfg, hidden_states, attention_mask, Wq, Wk, Wv, Wo, n_cores=8):
    """Build the 8 per-core input dicts (host-side prep: transpose + bf16)."""
    B = hidden_states.shape[0]
    gpc = n_cores // B  # cores per batch element
    wq_t = np.ascontiguousarray(Wq.T.astype(NP_BF16))
    wk_t = np.ascontiguousarray(Wk.T.astype(NP_BF16))
    wv_t = np.ascontiguousarray(Wv.T.astype(NP_BF16))
    wo_t = np.ascontiguousarray(Wo.T.astype(NP_BF16))
    in_maps = []
    for c in range(n_cores):
        b, g = c // gpc, c % gpc
        q0 = g * cfg.QPC
        hsq_c = np.ascontiguousarray(
            hidden_states[b, q0:q0 + cfg.QPC, :].T.astype(NP_BF16)
        )  # [H, QPC] — this core's query (= key-shard) columns
        msk = attention_mask[b, q0:q0 + cfg.QPC, :]  # [QPC, S]
        # The kernel processes keys in the order [own shard, peer g^1, g^2,
        # g^3] (own first from local tensors, peers via XOR-rotated gather
        # slots) — permute the mask's key axis to match.
        order = np.concatenate(
            [np.arange((g ^ j) * cfg.QPC, (g ^ j) * cfg.QPC + cfg.QPC)
             for j in range(gpc)]
        )
        mskT_c = np.ascontiguousarray(msk[:, order].T.astype(NP_BF16))
        in_maps.append(
            {
                "hsq": hsq_c,
                "wqt": wq_t,
                "wkt": wk_t,
                "wvt": wv_t,
                "wot": wo_t,
                "maskt": mskT_c,
            }
        )
    return in_maps


def assemble_output(cfg: Cfg, results, B, S, H, n_cores=8):
    out = np.empty((B, S, H), dtype=np.float32)
    gpc = n_cores // B
    for c in range(n_cores):
        b, g = c // gpc, c % gpc
        out[b, g * cfg.QPC:(g + 1) * cfg.QPC, :] = results[c]["o"]
    return out


_CACHED_NC = None


def kernel(hidden_states, attention_mask, Wq, Wk, Wv, Wo, **kw):
    global _CACHED_NC
    B, S, H = hidden_states.shape
    cfg = Cfg(S=S, H=H, QPC=(B * S) // 8)
    if _CACHED_NC is None:
        _CACHED_NC = build_nc(cfg)
    nc = _CACHED_NC
    in_maps = make_in_maps(cfg, np.asarray(hidden_states), np.asarray(attention_mask),
                           np.asarray(Wq), np.asarray(Wk), np.asarray(Wv), np.asarray(Wo))
    core_ids = list(range(8))
    last_exc = None
    for _ in range(3):  # the axon tunnel occasionally drops a worker
        try:
            res = run_bass_kernel_spmd(nc, in_maps, core_ids)
            return assemble_output(cfg, res.results, B, S, H)
        except Exception as e:  # noqa: BLE001
            last_exc = e
    raise last_exc
